# revision 35
# baseline (speedup 1.0000x reference)
"""Pairwise cosine-similarity kernel for Trainium2 (8 NeuronCores, SPMD).

Computes out = 16 * normalize(x1) @ normalize(x2).T for x1, x2 [8192, 512] f32.

Sharding: 2x4 grid. x1 rows split 2 ways (4096 rows/core), x2 rows split 4
ways (2048 rows/core); each core computes a [4096, 2048] output block; the
host assembles the grid and upcasts bf16 -> f32. The 2x4 split (vs the
earlier 4x2) halves the per-core x2-side norm/scale/broadcast work and
doubles the column-group window to 32 m-tiles, relieving the DVE/ACT
budget that was the binding constraint (measured 142.7us vs 143.4).

Host-side prep is layout/dtype only:
  - x1t [512, 4096] bf16: pre-transposed x1 slice (GEMM stationary source),
    loaded as 8 x [128, 2048] SBUF tiles (the proven tile shape).
  - x2t [512, 2048] bf16: pre-transposed x2 slice (GEMM moving source).
  - x1n [8, 128, 2048] fp8e4m3, x2n [4, 128, 2048] fp8e4m3: natural-layout
    row-grouped copies used only for row-norm computation
    (group g holds rows g*512 + j*128 + p at [g, p, j*512:(j+1)*512]).

All FLOPs run on device. Schedule notes (HW-measured 142.7us; the
prior-session baseline was 154.7us):
  1. Input DMA is deadline-ordered on the sync HWDGE queue: x2n g0/g1
     (norm stats feed the x2t scale chain), then x2t cg0, then all of x1t,
     then the rest. First bytes land ~8.7us after NEFF start (the engine
     preamble runs to ~6.9us and DMA can't start earlier).
  2. 30 junk fp16 matmuls fill 7.9-16us purely to lift the PE HAM clock
     gate (1.2 -> 2.4 GHz) before real work; they overlap the input DMA.
     Real GEMM matmuls start ~17us, paced by the x2t scale chain.
  3. Norm math: squares split ACT (j0/j1, Square+accum) / DVE (j2/j3,
     tensor_mul + tensor_reduce; tensor_tensor_reduce kills the device).
     inv1 folds the 16x output scale; inv2 is partition-broadcast via fp16
     diag matmuls (ones.T @ diag(inv2)). Stats for cg N+1 are emitted just
     below cg N's first evictions in priority so eviction latency (which
     gates PSUM bank recycling) stays low.
  4. PSUM: 6+2 single-bank [128, 512] tiles. Matmuls targeting offset
     slices of a 2-bank [128, 1024] PSUM tensor run but drop the whole
     chip to ~5/6 clock (as does chunking x1t into [128, 512] tiles) --
     keep matmul dst = whole single-bank tiles and x1t as 4 x [128, 2048].
  5. Matmuls run j-outer per m-tile so PSUM bank j0 is evicted (one
     [128, 512] op) while bank j1's matmuls still run; the evicting engine
     alternates with (j + m) parity. Out-DMA triggers issue from the sync
     queue; the last two m-tiles split across both engines and two DMAs
     each. A dummy early Sqrt preloads the ACT table (else a lazy 1.3us
     ACT_TABLE_LOAD lands on the stats critical path). Ten 4x2-grid
     schedule variants (DMA reorders, queue reassignment, deferred
     rescale) all measured slower; the DMA order here is deadline-driven
     and fragile -- re-measure any change.
"""

import sys

for _p in ("/root/.axon_site/_ro/trn_rl_repo", "/opt/trn_rl_repo"):
    if _p not in sys.path:
        sys.path.append(_p)

import ml_dtypes
import numpy as np

import concourse.bass as bass
import concourse.tile as tile
from concourse import bacc, mybir
from concourse.bass_utils import run_bass_kernel_spmd
from concourse.masks import make_identity

F32 = mybir.dt.float32
BF16 = mybir.dt.bfloat16
FP16 = mybir.dt.float16
FP8 = mybir.dt.float8e4
P = 128
SCALE = 16.0
EPS = 1e-8

N_CORES = 8
GRID_I = 2  # row-shards of x1
GRID_J = 4  # column-shards of x2
N1 = 8192
N2 = 8192
D = 512
CG = 1024  # output column-group width

_PROGRAM_CACHE = {}


def build_program(n1_local=N1 // GRID_I, n2=N2 // GRID_J, d=D):
    kc = d // P                 # 4 contraction chunks
    m_tiles = n1_local // P     # 16 row tiles per core
    n_cgs = n2 // CG            # 4 column groups
    g1 = n1_local // 512        # 4 x1 norm groups
    g2 = n2 // 512              # 8 x2 norm groups

    nc = bacc.Bacc("TRN2", target_bir_lowering=False, debug=False,
                   num_devices=N_CORES)
    x1t = nc.dram_tensor("x1t", [d, n1_local], BF16, kind="ExternalInput")
    x1n = nc.dram_tensor("x1n", [g1, P, 2048], FP8, kind="ExternalInput")
    x2n = nc.dram_tensor("x2n", [g2, P, 2048], FP8, kind="ExternalInput")
    x2t = nc.dram_tensor("x2t", [d, n2], BF16, kind="ExternalInput")
    out = nc.dram_tensor("out", [n1_local, n2], BF16, kind="ExternalOutput")

    AF = mybir.ActivationFunctionType
    ALU = mybir.AluOpType

    with tile.TileContext(nc) as tc:
        with (
            tc.tile_pool(name="const", bufs=1) as const,
            tc.tile_pool(name="xt", bufs=1) as xt,
            tc.tile_pool(name="ldn", bufs=1) as ldn,
            tc.tile_pool(name="sq", bufs=4) as sqp,
            tc.tile_pool(name="stat", bufs=1) as stat,
            tc.tile_pool(name="dg", bufs=2) as dgp,
            tc.tile_pool(name="bc", bufs=1) as bcp,
            tc.tile_pool(name="ot", bufs=6) as otp,
            tc.tile_pool(name="ps", bufs=6, space="PSUM") as psp,
            tc.tile_pool(name="psb", bufs=2, space="PSUM") as psb,
        ):
            # ---- constants (cheap memsets so PE warmup starts early) --------
            ones_h = const.tile([P, P], FP16)
            nc.gpsimd.memset(ones_h[:], 1.0)
            warm = const.tile([P, 512], FP16)
            nc.gpsimd.memset(warm[:], 0.0)
            ident4 = const.tile([P, 4, P], FP16)
            nc.gpsimd.memset(ident4[:], 0.0)
            for b in range(4):
                make_identity(nc, ident4[:, b], nomemset=True)
            # dummy sqrt pulls the ACT_TABLE_LOAD for Sqrt off the stats
            # critical path (it otherwise loads lazily, 1.3us, right before
            # the first inv2 sqrt).
            tblw = const.tile([P, 1], F32)
            nc.scalar.activation(tblw[:], ones_h[:, 0:1], AF.Sqrt)

            # ---- SBUF input tiles -------------------------------------------
            x1n_t = [ldn.tile([P, 4, 512], FP8, tag=f"x1n_{g}",
                              name=f"x1n_{g}") for g in range(g1)]
            x2n_t = [ldn.tile([P, 4, 512], FP8, tag=f"x2n_{g}",
                              name=f"x2n_{g}") for g in range(g2)]
            n_c = n1_local // 2048
            x1T = [[xt.tile([P, 2048], BF16, tag=f"x1T_{k}_{c}",
                            name=f"x1T_{k}_{c}") for c in range(n_c)]
                   for k in range(kc)]
            x2T = [[xt.tile([P, CG], BF16, tag=f"x2T_{k}_{cg}",
                            name=f"x2T_{k}_{cg}") for cg in range(n_cgs)]
                   for k in range(kc)]

            def dma_x2T(k, cg, eng=None):
                (eng or nc.sync).dma_start(
                    x2T[k][cg][:], x2t[k * P:(k + 1) * P,
                                       cg * CG:(cg + 1) * CG])

            def dma_x1T(k, c):
                nc.sync.dma_start(
                    x1T[k][c][:], x1t[k * P:(k + 1) * P,
                                      c * 2048:(c + 1) * 2048])

            def dma_x2n(g, eng=None):
                (eng or nc.sync).dma_start(
                    x2n_t[g][:], x2n.ap()[g].rearrange("p (j e) -> p j e", j=4)
                )

            def dma_x1n(g):
                nc.sync.dma_start(
                    x1n_t[g][:], x1n.ap()[g].rearrange("p (j e) -> p j e", j=4)
                )

            # ---- input DMAs (sync queue, deadline order) --------------------
            dma_x2n(0)
            dma_x2n(1)
            for k in range(kc):
                dma_x2T(k, 0)
            for k in range(kc):
                dma_x1T(k, 0)
            dma_x1n(0)
            dma_x1n(1)
            # x1T c1 isn't needed until m16 (~46us): mid-window norm
            # tensors cut ahead of it.
            dma_x1n(2)
            dma_x1n(3)
            for g in range(2, g2):
                dma_x2n(g)
            for k in range(kc):
                dma_x1T(k, 1)
            for g in range(4, g1):
                dma_x1n(g)
            for cg in range(1, n_cgs):
                for k in range(kc):
                    dma_x2T(k, cg)

            # ---- PE warmup against the HAM clock gate -----------------------
            for w in range(30):
                ps_w = psb.tile([P, 512], F32, tag="psb", name=f"warm_{w}")
                nc.tensor.matmul(ps_w[:], lhsT=ones_h[:], rhs=warm[:],
                                 start=True, stop=True)

            # ---- stats / broadcast helpers ----------------------------------
            ssq2 = [stat.tile([P, 8], F32, tag=f"ssq2_{cg}", name=f"ssq2_{cg}")
                    for cg in range(n_cgs)]
            inv2 = [stat.tile([P, 8], F32, tag=f"inv2_{cg}", name=f"inv2_{cg}")
                    for cg in range(n_cgs)]
            ssq1 = stat.tile([P, 4 * g1], F32, tag="ssq1")
            inv1 = stat.tile([P, 4 * g1], F32, tag="inv1")
            dg4s = {}
            psbs = {}
            bcs = [bcp.tile([P, CG], BF16, tag=f"bc_{cg}", name=f"bc_{cg}")
                   for cg in range(n_cgs)]

            def sq_j(src, acc, j, eng="act"):
                """acc[:, j] = row sum of src[:, j]^2 on the given engine."""
                if eng == "act":
                    sq_t = sqp.tile([P, 512], BF16, tag="sqa")
                    nc.scalar.activation(
                        sq_t[:], src[:, j], AF.Square,
                        accum_out=acc[:, j:j + 1],
                    )
                else:
                    sq_t = sqp.tile([P, 512], BF16, tag="sqv")
                    nc.vector.tensor_mul(sq_t[:], src[:, j], src[:, j])
                    nc.vector.tensor_reduce(
                        acc[:, j:j + 1], sq_t[:], op=ALU.add,
                        axis=mybir.AxisListType.X,
                    )

            def stats_x2_group(cg, h):
                """ssq2[cg][:, 4h:4h+4] from x2n group 2*cg+h (split engines)."""
                g = 2 * cg + h
                s = ssq2[cg]
                for j in range(4):
                    sq_j(x2n_t[g], s[:, 4 * h:4 * h + 4], j,
                         "act" if j < 2 else "dve")

            def inv2_finish(cg, h=None):
                """inv2[cg][half] = 1 / max(row_norm, EPS)."""
                sl = slice(0, 8) if h is None else slice(4 * h, 4 * h + 4)
                iv = inv2[cg][:, sl]
                nc.scalar.activation(iv, ssq2[cg][:, sl], AF.Sqrt)
                nc.vector.tensor_scalar_max(iv, iv, EPS)
                nc.vector.reciprocal(iv, iv)

            def stats_x1_group(g, engs=("act", "act", "act", "act")):
                """inv1[:, 4g:4g+4] = 16 / max(row_norm, EPS) (scale folded)."""
                for j in range(4):
                    sq_j(x1n_t[g], ssq1[:, 4 * g:4 * g + 4], j, engs[j])
                sl = slice(4 * g, 4 * g + 4)
                nc.scalar.activation(inv1[:, sl], ssq1[:, sl], AF.Sqrt,
                                     scale=1.0 / 256.0)
                nc.vector.tensor_scalar_max(inv1[:, sl], inv1[:, sl],
                                            EPS / 16.0)
                nc.vector.reciprocal(inv1[:, sl], inv1[:, sl])

            def dg4_build(cg, h):
                dg4 = dgp.tile([P, 4, P], FP16, tag="dg", name=f"dg_{cg}_{h}")
                nc.vector.tensor_mul(
                    dg4[:], ident4[:],
                    inv2[cg][:, 4 * h:4 * h + 4, None].to_broadcast((P, 4, P)),
                )
                dg4s[(cg, h)] = dg4

            def bcast_mm(cg, h):
                ps_b = psb.tile([P, 512], F32, tag="psb", name=f"psb_{cg}_{h}")
                nc.tensor.matmul(ps_b[:], lhsT=ones_h[:], rhs=dg4s[(cg, h)][:],
                                 start=True, stop=True)
                psbs[(cg, h)] = ps_b

            def bc_copy(cg, h):
                c0 = 4 * h * P
                nc.scalar.activation(bcs[cg][:, c0:c0 + 512],
                                     psbs[(cg, h)][:], AF.Copy)

            def scale_x2(cg, k, h=None):
                """x2T[k][cg] *= bcs[cg] in place (DVE, baseline-proven)."""
                sl = slice(0, CG) if h is None else slice(512 * h,
                                                          512 * (h + 1))
                nc.vector.tensor_mul(x2T[k][cg][:, sl], x2T[k][cg][:, sl],
                                     bcs[cg][:, sl])

            def gemm_m(cg, m):
                pss = [psp.tile([P, 512], F32, tag="ps",
                                name=f"ps_{cg}_{m}_{j}") for j in range(2)]
                ot = otp.tile([P, CG], BF16, tag="ot", name=f"ot_{cg}_{m}")
                iv = inv1[:, m:m + 1]
                last = (cg == n_cgs - 1) and (m >= m_tiles - 2)
                # j-outer: finish bank j0's accumulation first, evict it on
                # DVE while bank j1's matmuls run, then evict j1 on ACT.
                # Each engine does one [P, 512] per m-tile -> banks recycle
                # ~2x faster than both-halves-on-one-engine.
                for j in range(2):
                    for k in range(kc):
                        nc.tensor.matmul(
                            pss[j][:],
                            lhsT=x1T[k][m // 16][:, (m % 16) * P:
                                                 (m % 16 + 1) * P],
                            rhs=x2T[k][cg][:, j * 512:(j + 1) * 512],
                            start=(k == 0), stop=(k == kc - 1),
                        )
                    sl = slice(j * 512, (j + 1) * 512)
                    if (j + m) % 2 == 0:
                        nc.vector.tensor_scalar_mul(ot[:, sl], pss[j][:], iv)
                    else:
                        nc.scalar.activation(ot[:, sl], pss[j][:],
                                             AF.Copy, scale=iv)
                if last:
                    base = cg * CG
                    nc.sync.dma_start(
                        out.ap()[m * P:(m + 1) * P, base:base + 512],
                        ot[:, 0:512])
                    nc.sync.dma_start(
                        out.ap()[m * P:(m + 1) * P, base + 512:base + 1024],
                        ot[:, 512:1024])
                else:
                    nc.sync.dma_start(
                        out.ap()[m * P:(m + 1) * P, cg * CG:(cg + 1) * CG],
                        ot[:])

            # ---- bootstrap stats for cg0 (overlaps warmup + first DMAs) -----
            # Program order defines RAW deps in the online Tile tracker, so
            # everything the cg0 evictions read (bcs[0], inv1 g0) must be
            # emitted before gemm_m(0, 0). The PE's gemm matmuls don't depend
            # on any of it, so the scheduler still overlaps them.
            stats_x2_group(0, 0)
            inv2_finish(0, 0)
            dg4_build(0, 0)
            bcast_mm(0, 0)
            bc_copy(0, 0)
            stats_x2_group(0, 1)
            inv2_finish(0, 1)
            dg4_build(0, 1)
            bcast_mm(0, 1)
            bc_copy(0, 1)
            for k in range(kc):
                scale_x2(0, k, 0)
            for k in range(kc):
                scale_x2(0, k, 1)
            stats_x1_group(0)

            # ---- main loop --------------------------------------------------
            # per cg: 16 m-tiles; bcast matmuls for cg0 slot in after m0 (their
            # diag inputs are ready by then); stats/broadcast/scale for cg+1
            # are spread through the window.
            for cg in range(n_cgs):
                nxt = cg + 1
                for m in range(m_tiles):
                    gemm_m(cg, m)
                    if cg == 0:
                        # x1 norm group g is needed by m-tile 4g's eviction;
                        # one group per ~7us keeps the engines unclustered.
                        # cg1 stats sit at m==9/11/13 (deadline m==25) so
                        # their DVE ops never collide with the m0-m5
                        # evictions that gate PSUM recycling.
                        if m == 0:
                            stats_x1_group(1)
                        elif m == 9:
                            stats_x2_group(1, 0)
                        elif m == 11:
                            stats_x2_group(1, 1)
                            inv2_finish(1)
                        elif m == 13:
                            dg4_build(1, 0)
                            dg4_build(1, 1)
                        elif m in (6, 10, 14, 18, 22, 26):
                            stats_x1_group(2 + (m - 6) // 4,
                                           ("act", "act", "dve", "dve")
                                           if m in (10, 14, 18, 22) else
                                           ("act", "act", "act", "act"))
                    if nxt < n_cgs:
                        if m == 25:
                            bcast_mm(nxt, 0)
                            bc_copy(nxt, 0)
                        elif m == 27:
                            bcast_mm(nxt, 1)
                            bc_copy(nxt, 1)
                            for k in range(kc):
                                scale_x2(nxt, k)

    nc.compile()
    return nc


def _get_program():
    key = "default"
    if key not in _PROGRAM_CACHE:
        _PROGRAM_CACHE[key] = build_program()
    return _PROGRAM_CACHE[key]


def _norm_groups(x8: np.ndarray) -> np.ndarray:
    """[G*512, 512] f32 -> [G, 128, 2048] fp8 with rows g*512+j*128+p."""
    g = x8.shape[0] // 512
    r = x8.reshape(g, 4, P, 512).transpose(0, 2, 1, 3).reshape(g, P, 2048)
    return np.ascontiguousarray(r.astype(ml_dtypes.float8_e4m3))


def make_in_maps(x1: np.ndarray, x2: np.ndarray) -> list:
    x1 = np.asarray(x1, dtype=np.float32)
    x2 = np.asarray(x2, dtype=np.float32)
    assert x1.shape == (N1, D) and x2.shape == (N2, D), (x1.shape, x2.shape)
    x1_b = x1.astype(ml_dtypes.bfloat16)
    x2_b = x2.astype(ml_dtypes.bfloat16)
    rows = N1 // GRID_I
    cols = N2 // GRID_J
    x1t_i = [np.ascontiguousarray(x1_b[i * rows:(i + 1) * rows].T)
             for i in range(GRID_I)]
    x1n_i = [_norm_groups(x1[i * rows:(i + 1) * rows]) for i in range(GRID_I)]
    x2t_j = [np.ascontiguousarray(x2_b[j * cols:(j + 1) * cols].T)
             for j in range(GRID_J)]
    x2n_j = [_norm_groups(x2[j * cols:(j + 1) * cols]) for j in range(GRID_J)]
    maps = []
    for c in range(N_CORES):
        i, j = c // GRID_J, c % GRID_J
        maps.append({
            "x1t": x1t_i[i],
            "x1n": x1n_i[i],
            "x2n": x2n_j[j],
            "x2t": x2t_j[j],
        })
    return maps


def kernel(x1: np.ndarray, x2: np.ndarray) -> np.ndarray:
    nc = _get_program()
    in_maps = make_in_maps(x1, x2)
    res = run_bass_kernel_spmd(nc, in_maps, core_ids=list(range(N_CORES)))
    rows = N1 // GRID_I
    cols = N2 // GRID_J
    full = np.empty((N1, N2), dtype=np.float32)
    for c in range(N_CORES):
        i, j = c // GRID_J, c % GRID_J
        full[i * rows:(i + 1) * rows, j * cols:(j + 1) * cols] = \
            res.results[c]["out"]
    return full


if __name__ == "__main__":
    rng = np.random.default_rng(0)
    a = rng.standard_normal((N1, D), dtype=np.float32)
    b = rng.standard_normal((N2, D), dtype=np.float32)
    got = kernel(a, b)
    n1 = np.maximum(np.linalg.norm(a, axis=-1, keepdims=True), EPS)
    n2 = np.maximum(np.linalg.norm(b, axis=-1, keepdims=True), EPS)
    want = SCALE * (a / n1) @ (b / n2).T
    err = np.abs(got - want)
    rel = np.linalg.norm(got - want) / np.linalg.norm(want)
    print(f"max abs err: {err.max():.3e}  rel: {rel:.3e}")


# revision 39
# speedup vs baseline: 1.0047x; 1.0047x over previous
"""Pairwise cosine-similarity kernel for Trainium2 (8 NeuronCores, SPMD).

Computes out = 16 * normalize(x1) @ normalize(x2).T for x1, x2 [8192, 512] f32.

Sharding: 4x2 grid. x1 rows split 4 ways (2048 rows/core), x2 rows split 2
ways (4096 rows/core); each core computes a [2048, 4096] output block; the
host concatenates and upcasts bf16 -> f32.

Host-side prep is layout/dtype only:
  - x1t [512, 2048] bf16: pre-transposed x1 slice (GEMM stationary source).
  - x2t [512, 4096] bf16: pre-transposed x2 slice (GEMM moving source).
  - x1n [4, 128, 2048] fp8e4m3, x2n [8, 128, 2048] fp8e4m3: natural-layout
    row-grouped copies used only for row-norm computation
    (group g holds rows g*512 + j*128 + p at [g, p, j*512:(j+1)*512]).

All FLOPs run on device. Schedule notes (HW-measured, ~146us vs 155us for
the previous version of this kernel):
  1. Input DMA is deadline-ordered on the sync HWDGE queue: x2n g0/g1
     (norm stats feed the x2t scale chain), then x2t cg0, then all of x1t,
     then the rest. First bytes land ~8.7us after NEFF start (the engine
     preamble runs to ~6.9us and DMA can't start earlier).
  2. 30 junk fp16 matmuls fill 7.9-16us purely to lift the PE HAM clock
     gate (1.2 -> 2.4 GHz) before real work; they overlap the input DMA.
     Real GEMM matmuls start ~22us, paced by the x2t scale chain.
  3. Norm math: squares split ACT (j0/j1, Square+accum) / DVE (j2/j3,
     tensor_mul + tensor_reduce; tensor_tensor_reduce kills the device).
     inv1 folds the 16x output scale; inv2 is partition-broadcast via fp16
     diag matmuls (ones.T @ diag(inv2)). Stats for cg N+1 are emitted just
     below cg N's first evictions in priority so eviction latency (which
     gates PSUM bank recycling) stays low.
  4. PSUM: 6+2 single-bank [128, 512] tiles. Matmuls targeting offset
     slices of a 2-bank [128, 1024] PSUM tensor run but drop the whole
     chip to ~5/6 clock (as does chunking x1t into [128, 512] tiles) --
     keep matmul dst = whole single-bank tiles and x1t as 4 x [128, 2048].
  5. Evictions alternate DVE tensor_scalar / ACT activation(Copy, scale)
     per m-tile, one [128, 512] op per PSUM bank; out-DMA triggers issue
     from the sync queue; the last two m-tiles split across both engines
     and two DMAs each to shorten the tail.
"""

import sys

for _p in ("/root/.axon_site/_ro/trn_rl_repo", "/opt/trn_rl_repo"):
    if _p not in sys.path:
        sys.path.append(_p)

import ml_dtypes
import numpy as np

import concourse.bass as bass
import concourse.tile as tile
from concourse import bacc, mybir
from concourse.bass_utils import run_bass_kernel_spmd
from concourse.masks import make_identity

F32 = mybir.dt.float32
BF16 = mybir.dt.bfloat16
FP16 = mybir.dt.float16
FP8 = mybir.dt.float8e4
P = 128
SCALE = 16.0
EPS = 1e-8

N_CORES = 8
GRID_I = 2  # row-shards of x1
GRID_J = 4  # column-shards of x2
N1 = 8192
N2 = 8192
D = 512
CG = 1024  # output column-group width

_PROGRAM_CACHE = {}


def build_program(n1_local=N1 // GRID_I, n2=N2 // GRID_J, d=D):
    kc = d // P                 # 4 contraction chunks
    m_tiles = n1_local // P     # 16 row tiles per core
    n_cgs = n2 // CG            # 4 column groups
    g1 = n1_local // 512        # 4 x1 norm groups
    g2 = n2 // 512              # 8 x2 norm groups

    nc = bacc.Bacc("TRN2", target_bir_lowering=False, debug=False,
                   num_devices=N_CORES)
    x1t = nc.dram_tensor("x1t", [d, n1_local], BF16, kind="ExternalInput")
    x1n = nc.dram_tensor("x1n", [g1, P, 2048], FP8, kind="ExternalInput")
    x2n = nc.dram_tensor("x2n", [g2, P, 2048], FP8, kind="ExternalInput")
    x2t = nc.dram_tensor("x2t", [d, n2], BF16, kind="ExternalInput")
    out = nc.dram_tensor("out", [n1_local, n2], BF16, kind="ExternalOutput")

    AF = mybir.ActivationFunctionType
    ALU = mybir.AluOpType

    with tile.TileContext(nc) as tc:
        with (
            tc.tile_pool(name="const", bufs=1) as const,
            tc.tile_pool(name="xt", bufs=1) as xt,
            tc.tile_pool(name="ldn", bufs=1) as ldn,
            tc.tile_pool(name="sq", bufs=4) as sqp,
            tc.tile_pool(name="stat", bufs=1) as stat,
            tc.tile_pool(name="dg", bufs=2) as dgp,
            tc.tile_pool(name="bc", bufs=1) as bcp,
            tc.tile_pool(name="ot", bufs=6) as otp,
            tc.tile_pool(name="ps", bufs=6, space="PSUM") as psp,
            tc.tile_pool(name="psb", bufs=2, space="PSUM") as psb,
        ):
            # ---- constants (cheap memsets so PE warmup starts early) --------
            ones_h = const.tile([P, P], FP16)
            nc.gpsimd.memset(ones_h[:], 1.0)
            warm = const.tile([P, 512], FP16)
            nc.gpsimd.memset(warm[:], 0.0)
            ident4 = const.tile([P, 4, P], FP16)
            nc.gpsimd.memset(ident4[:], 0.0)
            for b in range(4):
                make_identity(nc, ident4[:, b], nomemset=True)
            # dummy sqrt pulls the ACT_TABLE_LOAD for Sqrt off the stats
            # critical path (it otherwise loads lazily, 1.3us, right before
            # the first inv2 sqrt).
            tblw = const.tile([P, 1], F32)
            nc.scalar.activation(tblw[:], ones_h[:, 0:1], AF.Sqrt)

            # ---- SBUF input tiles -------------------------------------------
            x1n_t = [ldn.tile([P, 4, 512], FP8, tag=f"x1n_{g}",
                              name=f"x1n_{g}") for g in range(g1)]
            x2n_t = [ldn.tile([P, 4, 512], FP8, tag=f"x2n_{g}",
                              name=f"x2n_{g}") for g in range(g2)]
            n_c = n1_local // 2048
            x1T = [[xt.tile([P, 2048], BF16, tag=f"x1T_{k}_{c}",
                            name=f"x1T_{k}_{c}") for c in range(n_c)]
                   for k in range(kc)]
            x2T = [[xt.tile([P, CG], BF16, tag=f"x2T_{k}_{cg}",
                            name=f"x2T_{k}_{cg}") for cg in range(n_cgs)]
                   for k in range(kc)]

            def dma_x2T(k, cg, eng=None):
                (eng or nc.sync).dma_start(
                    x2T[k][cg][:], x2t[k * P:(k + 1) * P,
                                       cg * CG:(cg + 1) * CG])

            def dma_x1T(k, c):
                nc.sync.dma_start(
                    x1T[k][c][:], x1t[k * P:(k + 1) * P,
                                      c * 2048:(c + 1) * 2048])

            def dma_x2n(g, eng=None):
                (eng or nc.sync).dma_start(
                    x2n_t[g][:], x2n.ap()[g].rearrange("p (j e) -> p j e", j=4)
                )

            def dma_x1n(g):
                nc.sync.dma_start(
                    x1n_t[g][:], x1n.ap()[g].rearrange("p (j e) -> p j e", j=4)
                )

            # ---- input DMAs (sync queue, deadline order) --------------------
            dma_x2n(0)
            dma_x2n(1)
            for k in range(kc):
                dma_x2T(k, 0)
            for k in range(kc):
                dma_x1T(k, 0)
            dma_x1n(0)
            dma_x1n(1)
            # x1n g2/g3 cut ahead of x1T c1 (its deadline is m16, ~46us)
            # so inv1-g2/g3 beat their m8/m12 eviction deadlines; x2n g2/g3
            # stay behind c1 so cg1's stats don't fire amid early evictions
            # (v21 moved both and its gain/loss cancelled).
            dma_x1n(2)
            dma_x1n(3)
            for k in range(kc):
                dma_x1T(k, 1)
            for g in range(4, g1):
                dma_x1n(g)
            for g in range(2, g2):
                dma_x2n(g)
            for cg in range(1, n_cgs):
                for k in range(kc):
                    dma_x2T(k, cg)

            # ---- PE warmup against the HAM clock gate -----------------------
            for w in range(30):
                ps_w = psb.tile([P, 512], F32, tag="psb", name=f"warm_{w}")
                nc.tensor.matmul(ps_w[:], lhsT=ones_h[:], rhs=warm[:],
                                 start=True, stop=True)

            # ---- stats / broadcast helpers ----------------------------------
            ssq2 = [stat.tile([P, 8], F32, tag=f"ssq2_{cg}", name=f"ssq2_{cg}")
                    for cg in range(n_cgs)]
            inv2 = [stat.tile([P, 8], F32, tag=f"inv2_{cg}", name=f"inv2_{cg}")
                    for cg in range(n_cgs)]
            ssq1 = stat.tile([P, 4 * g1], F32, tag="ssq1")
            inv1 = stat.tile([P, 4 * g1], F32, tag="inv1")
            dg4s = {}
            psbs = {}
            bcs = [bcp.tile([P, CG], BF16, tag=f"bc_{cg}", name=f"bc_{cg}")
                   for cg in range(n_cgs)]

            def sq_j(src, acc, j, eng="act"):
                """acc[:, j] = row sum of src[:, j]^2 on the given engine."""
                if eng == "act":
                    sq_t = sqp.tile([P, 512], BF16, tag="sqa")
                    nc.scalar.activation(
                        sq_t[:], src[:, j], AF.Square,
                        accum_out=acc[:, j:j + 1],
                    )
                else:
                    sq_t = sqp.tile([P, 512], BF16, tag="sqv")
                    nc.vector.tensor_mul(sq_t[:], src[:, j], src[:, j])
                    nc.vector.tensor_reduce(
                        acc[:, j:j + 1], sq_t[:], op=ALU.add,
                        axis=mybir.AxisListType.X,
                    )

            def stats_x2_group(cg, h):
                """ssq2[cg][:, 4h:4h+4] from x2n group 2*cg+h (split engines)."""
                g = 2 * cg + h
                s = ssq2[cg]
                for j in range(4):
                    sq_j(x2n_t[g], s[:, 4 * h:4 * h + 4], j,
                         "act" if j < 2 else "dve")

            def inv2_finish(cg, h=None):
                """inv2[cg][half] = 1 / max(row_norm, EPS)."""
                sl = slice(0, 8) if h is None else slice(4 * h, 4 * h + 4)
                iv = inv2[cg][:, sl]
                nc.scalar.activation(iv, ssq2[cg][:, sl], AF.Sqrt)
                nc.vector.tensor_scalar_max(iv, iv, EPS)
                nc.vector.reciprocal(iv, iv)

            def stats_x1_group(g, engs=("act", "act", "act", "act")):
                """inv1[:, 4g:4g+4] = 16 / max(row_norm, EPS) (scale folded)."""
                for j in range(4):
                    sq_j(x1n_t[g], ssq1[:, 4 * g:4 * g + 4], j, engs[j])
                sl = slice(4 * g, 4 * g + 4)
                nc.scalar.activation(inv1[:, sl], ssq1[:, sl], AF.Sqrt,
                                     scale=1.0 / 256.0)
                nc.vector.tensor_scalar_max(inv1[:, sl], inv1[:, sl],
                                            EPS / 16.0)
                nc.vector.reciprocal(inv1[:, sl], inv1[:, sl])

            def dg4_build(cg, h):
                dg4 = dgp.tile([P, 4, P], FP16, tag="dg", name=f"dg_{cg}_{h}")
                nc.vector.tensor_mul(
                    dg4[:], ident4[:],
                    inv2[cg][:, 4 * h:4 * h + 4, None].to_broadcast((P, 4, P)),
                )
                dg4s[(cg, h)] = dg4

            def bcast_mm(cg, h):
                ps_b = psb.tile([P, 512], F32, tag="psb", name=f"psb_{cg}_{h}")
                nc.tensor.matmul(ps_b[:], lhsT=ones_h[:], rhs=dg4s[(cg, h)][:],
                                 start=True, stop=True)
                psbs[(cg, h)] = ps_b

            def bc_copy(cg, h):
                c0 = 4 * h * P
                nc.scalar.activation(bcs[cg][:, c0:c0 + 512],
                                     psbs[(cg, h)][:], AF.Copy)

            def scale_x2(cg, k, h=None):
                """x2T[k][cg] *= bcs[cg] in place (DVE, baseline-proven)."""
                sl = slice(0, CG) if h is None else slice(512 * h,
                                                          512 * (h + 1))
                nc.vector.tensor_mul(x2T[k][cg][:, sl], x2T[k][cg][:, sl],
                                     bcs[cg][:, sl])

            def gemm_m(cg, m):
                pss = [psp.tile([P, 512], F32, tag="ps",
                                name=f"ps_{cg}_{m}_{j}") for j in range(2)]
                ot = otp.tile([P, CG], BF16, tag="ot", name=f"ot_{cg}_{m}")
                iv = inv1[:, m:m + 1]
                last = (cg == n_cgs - 1) and (m >= m_tiles - 2)
                # j-outer: finish bank j0's accumulation first, evict it on
                # DVE while bank j1's matmuls run, then evict j1 on ACT.
                # Each engine does one [P, 512] per m-tile -> banks recycle
                # ~2x faster than both-halves-on-one-engine.
                for j in range(2):
                    for k in range(kc):
                        nc.tensor.matmul(
                            pss[j][:],
                            lhsT=x1T[k][m // 16][:, (m % 16) * P:
                                                 (m % 16 + 1) * P],
                            rhs=x2T[k][cg][:, j * 512:(j + 1) * 512],
                            start=(k == 0), stop=(k == kc - 1),
                        )
                    sl = slice(j * 512, (j + 1) * 512)
                    if (j + m) % 2 == 0:
                        nc.vector.tensor_scalar_mul(ot[:, sl], pss[j][:], iv)
                    else:
                        nc.scalar.activation(ot[:, sl], pss[j][:],
                                             AF.Copy, scale=iv)
                if last:
                    base = cg * CG
                    nc.sync.dma_start(
                        out.ap()[m * P:(m + 1) * P, base:base + 512],
                        ot[:, 0:512])
                    nc.sync.dma_start(
                        out.ap()[m * P:(m + 1) * P, base + 512:base + 1024],
                        ot[:, 512:1024])
                else:
                    nc.sync.dma_start(
                        out.ap()[m * P:(m + 1) * P, cg * CG:(cg + 1) * CG],
                        ot[:])

            # ---- bootstrap stats for cg0 (overlaps warmup + first DMAs) -----
            # Program order defines RAW deps in the online Tile tracker, so
            # everything the cg0 evictions read (bcs[0], inv1 g0) must be
            # emitted before gemm_m(0, 0). The PE's gemm matmuls don't depend
            # on any of it, so the scheduler still overlaps them.
            stats_x2_group(0, 0)
            inv2_finish(0, 0)
            dg4_build(0, 0)
            bcast_mm(0, 0)
            bc_copy(0, 0)
            stats_x2_group(0, 1)
            inv2_finish(0, 1)
            dg4_build(0, 1)
            bcast_mm(0, 1)
            bc_copy(0, 1)
            for k in range(kc):
                scale_x2(0, k, 0)
            for k in range(kc):
                scale_x2(0, k, 1)
            stats_x1_group(0)

            # ---- main loop --------------------------------------------------
            # per cg: 16 m-tiles; bcast matmuls for cg0 slot in after m0 (their
            # diag inputs are ready by then); stats/broadcast/scale for cg+1
            # are spread through the window.
            for cg in range(n_cgs):
                nxt = cg + 1
                for m in range(m_tiles):
                    gemm_m(cg, m)
                    if cg == 0:
                        # x1 norm group g is needed by m-tile 4g's eviction;
                        # one group per ~7us keeps the engines unclustered.
                        if m == 0:
                            stats_x1_group(1)
                        elif m == 3:
                            stats_x2_group(1, 0)
                        elif m == 5:
                            stats_x2_group(1, 1)
                            inv2_finish(1)
                        elif m == 7:
                            dg4_build(1, 0)
                            dg4_build(1, 1)
                        elif m in (6, 10, 14, 18, 22, 26):
                            stats_x1_group(2 + (m - 6) // 4,
                                           ("act", "act", "dve", "dve")
                                           if m in (10, 18) else
                                           ("act", "act", "act", "act"))
                    if nxt < n_cgs:
                        if m == 25:
                            bcast_mm(nxt, 0)
                            bc_copy(nxt, 0)
                        elif m == 27:
                            bcast_mm(nxt, 1)
                            bc_copy(nxt, 1)
                            for k in range(kc):
                                scale_x2(nxt, k)

    nc.compile()
    return nc


def _get_program():
    key = "default"
    if key not in _PROGRAM_CACHE:
        _PROGRAM_CACHE[key] = build_program()
    return _PROGRAM_CACHE[key]


def _norm_groups(x8: np.ndarray) -> np.ndarray:
    """[G*512, 512] f32 -> [G, 128, 2048] fp8 with rows g*512+j*128+p."""
    g = x8.shape[0] // 512
    r = x8.reshape(g, 4, P, 512).transpose(0, 2, 1, 3).reshape(g, P, 2048)
    return np.ascontiguousarray(r.astype(ml_dtypes.float8_e4m3))


def make_in_maps(x1: np.ndarray, x2: np.ndarray) -> list:
    x1 = np.asarray(x1, dtype=np.float32)
    x2 = np.asarray(x2, dtype=np.float32)
    assert x1.shape == (N1, D) and x2.shape == (N2, D), (x1.shape, x2.shape)
    x1_b = x1.astype(ml_dtypes.bfloat16)
    x2_b = x2.astype(ml_dtypes.bfloat16)
    rows = N1 // GRID_I
    cols = N2 // GRID_J
    x1t_i = [np.ascontiguousarray(x1_b[i * rows:(i + 1) * rows].T)
             for i in range(GRID_I)]
    x1n_i = [_norm_groups(x1[i * rows:(i + 1) * rows]) for i in range(GRID_I)]
    x2t_j = [np.ascontiguousarray(x2_b[j * cols:(j + 1) * cols].T)
             for j in range(GRID_J)]
    x2n_j = [_norm_groups(x2[j * cols:(j + 1) * cols]) for j in range(GRID_J)]
    maps = []
    for c in range(N_CORES):
        i, j = c // GRID_J, c % GRID_J
        maps.append({
            "x1t": x1t_i[i],
            "x1n": x1n_i[i],
            "x2n": x2n_j[j],
            "x2t": x2t_j[j],
        })
    return maps


def kernel(x1: np.ndarray, x2: np.ndarray) -> np.ndarray:
    nc = _get_program()
    in_maps = make_in_maps(x1, x2)
    res = run_bass_kernel_spmd(nc, in_maps, core_ids=list(range(N_CORES)))
    rows = N1 // GRID_I
    cols = N2 // GRID_J
    full = np.empty((N1, N2), dtype=np.float32)
    for c in range(N_CORES):
        i, j = c // GRID_J, c % GRID_J
        full[i * rows:(i + 1) * rows, j * cols:(j + 1) * cols] = \
            res.results[c]["out"]
    return full


if __name__ == "__main__":
    rng = np.random.default_rng(0)
    a = rng.standard_normal((N1, D), dtype=np.float32)
    b = rng.standard_normal((N2, D), dtype=np.float32)
    got = kernel(a, b)
    n1 = np.maximum(np.linalg.norm(a, axis=-1, keepdims=True), EPS)
    n2 = np.maximum(np.linalg.norm(b, axis=-1, keepdims=True), EPS)
    want = SCALE * (a / n1) @ (b / n2).T
    err = np.abs(got - want)
    rel = np.linalg.norm(got - want) / np.linalg.norm(want)
    print(f"max abs err: {err.max():.3e}  rel: {rel:.3e}")


# revision 41
# speedup vs baseline: 1.0125x; 1.0078x over previous
"""Pairwise cosine-similarity kernel for Trainium2 (8 NeuronCores, SPMD).

Computes out = 16 * normalize(x1) @ normalize(x2).T for x1, x2 [8192, 512] f32.

Sharding: 4x2 grid. x1 rows split 4 ways (2048 rows/core), x2 rows split 2
ways (4096 rows/core); each core computes a [2048, 4096] output block; the
host concatenates and upcasts bf16 -> f32.

Host-side prep is layout/dtype only:
  - x1t [512, 2048] bf16: pre-transposed x1 slice (GEMM stationary source).
  - x2t [512, 4096] bf16: pre-transposed x2 slice (GEMM moving source).
  - x1n [4, 128, 2048] fp8e4m3, x2n [8, 128, 2048] fp8e4m3: natural-layout
    row-grouped copies used only for row-norm computation
    (group g holds rows g*512 + j*128 + p at [g, p, j*512:(j+1)*512]).

All FLOPs run on device. Schedule notes (HW-measured, ~146us vs 155us for
the previous version of this kernel):
  1. Input DMA is deadline-ordered on the sync HWDGE queue: x2n g0/g1
     (norm stats feed the x2t scale chain), then x2t cg0, then all of x1t,
     then the rest. First bytes land ~8.7us after NEFF start (the engine
     preamble runs to ~6.9us and DMA can't start earlier).
  2. 30 junk fp16 matmuls fill 7.9-16us purely to lift the PE HAM clock
     gate (1.2 -> 2.4 GHz) before real work; they overlap the input DMA.
     Real GEMM matmuls start ~22us, paced by the x2t scale chain.
  3. Norm math: squares split ACT (j0/j1, Square+accum) / DVE (j2/j3,
     tensor_mul + tensor_reduce; tensor_tensor_reduce kills the device).
     inv1 folds the 16x output scale; inv2 is partition-broadcast via fp16
     diag matmuls (ones.T @ diag(inv2)). Stats for cg N+1 are emitted just
     below cg N's first evictions in priority so eviction latency (which
     gates PSUM bank recycling) stays low.
  4. PSUM: 6+2 single-bank [128, 512] tiles. Matmuls targeting offset
     slices of a 2-bank [128, 1024] PSUM tensor run but drop the whole
     chip to ~5/6 clock (as does chunking x1t into [128, 512] tiles) --
     keep matmul dst = whole single-bank tiles and x1t as 4 x [128, 2048].
  5. Evictions alternate DVE tensor_scalar / ACT activation(Copy, scale)
     per m-tile, one [128, 512] op per PSUM bank; out-DMA triggers issue
     from the sync queue; the last two m-tiles split across both engines
     and two DMAs each to shorten the tail.
"""

import sys

for _p in ("/root/.axon_site/_ro/trn_rl_repo", "/opt/trn_rl_repo"):
    if _p not in sys.path:
        sys.path.append(_p)

import ml_dtypes
import numpy as np

import concourse.bass as bass
import concourse.tile as tile
from concourse import bacc, mybir
from concourse.bass_utils import run_bass_kernel_spmd
from concourse.masks import make_identity

F32 = mybir.dt.float32
BF16 = mybir.dt.bfloat16
FP16 = mybir.dt.float16
FP8 = mybir.dt.float8e4
P = 128
SCALE = 16.0
EPS = 1e-8

N_CORES = 8
GRID_I = 2  # row-shards of x1
GRID_J = 4  # column-shards of x2
N1 = 8192
N2 = 8192
D = 512
CG = 1024  # output column-group width

_PROGRAM_CACHE = {}


def build_program(n1_local=N1 // GRID_I, n2=N2 // GRID_J, d=D):
    kc = d // P                 # 4 contraction chunks
    m_tiles = n1_local // P     # 16 row tiles per core
    n_cgs = n2 // CG            # 4 column groups
    g1 = n1_local // 512        # 4 x1 norm groups
    g2 = n2 // 512              # 8 x2 norm groups

    nc = bacc.Bacc("TRN2", target_bir_lowering=False, debug=False,
                   num_devices=N_CORES)
    x1t = nc.dram_tensor("x1t", [d, n1_local], BF16, kind="ExternalInput")
    x1n = nc.dram_tensor("x1n", [g1, P, 2048], FP8, kind="ExternalInput")
    x2n = nc.dram_tensor("x2n", [g2, P, 2048], FP8, kind="ExternalInput")
    x2t = nc.dram_tensor("x2t", [d, n2], BF16, kind="ExternalInput")
    out = nc.dram_tensor("out", [n1_local, n2], BF16, kind="ExternalOutput")

    AF = mybir.ActivationFunctionType
    ALU = mybir.AluOpType

    with tile.TileContext(nc) as tc:
        with (
            tc.tile_pool(name="const", bufs=1) as const,
            tc.tile_pool(name="xt", bufs=1) as xt,
            tc.tile_pool(name="ldn", bufs=1) as ldn,
            tc.tile_pool(name="sq", bufs=4) as sqp,
            tc.tile_pool(name="stat", bufs=1) as stat,
            tc.tile_pool(name="dg", bufs=2) as dgp,
            tc.tile_pool(name="bc", bufs=1) as bcp,
            tc.tile_pool(name="ot", bufs=8) as otp,
            tc.tile_pool(name="ps", bufs=6, space="PSUM") as psp,
            tc.tile_pool(name="psb", bufs=2, space="PSUM") as psb,
        ):
            # ---- constants (cheap memsets so PE warmup starts early) --------
            ones_h = const.tile([P, P], FP16)
            nc.gpsimd.memset(ones_h[:], 1.0)
            warm = const.tile([P, 512], FP16)
            nc.gpsimd.memset(warm[:], 0.0)
            ident4 = const.tile([P, 4, P], FP16)
            nc.gpsimd.memset(ident4[:], 0.0)
            for b in range(4):
                make_identity(nc, ident4[:, b], nomemset=True)
            # dummy sqrt pulls the ACT_TABLE_LOAD for Sqrt off the stats
            # critical path (it otherwise loads lazily, 1.3us, right before
            # the first inv2 sqrt).
            tblw = const.tile([P, 1], F32)
            nc.scalar.activation(tblw[:], ones_h[:, 0:1], AF.Sqrt)

            # ---- SBUF input tiles -------------------------------------------
            x1n_t = [ldn.tile([P, 4, 512], FP8, tag=f"x1n_{g}",
                              name=f"x1n_{g}") for g in range(g1)]
            x2n_t = [ldn.tile([P, 4, 512], FP8, tag=f"x2n_{g}",
                              name=f"x2n_{g}") for g in range(g2)]
            n_c = n1_local // 2048
            x1T = [[xt.tile([P, 2048], BF16, tag=f"x1T_{k}_{c}",
                            name=f"x1T_{k}_{c}") for c in range(n_c)]
                   for k in range(kc)]
            x2T = [[xt.tile([P, CG], BF16, tag=f"x2T_{k}_{cg}",
                            name=f"x2T_{k}_{cg}") for cg in range(n_cgs)]
                   for k in range(kc)]

            def dma_x2T(k, cg, eng=None):
                (eng or nc.sync).dma_start(
                    x2T[k][cg][:], x2t[k * P:(k + 1) * P,
                                       cg * CG:(cg + 1) * CG])

            def dma_x1T(k, c):
                nc.sync.dma_start(
                    x1T[k][c][:], x1t[k * P:(k + 1) * P,
                                      c * 2048:(c + 1) * 2048])

            def dma_x2n(g, eng=None):
                (eng or nc.sync).dma_start(
                    x2n_t[g][:], x2n.ap()[g].rearrange("p (j e) -> p j e", j=4)
                )

            def dma_x1n(g):
                nc.sync.dma_start(
                    x1n_t[g][:], x1n.ap()[g].rearrange("p (j e) -> p j e", j=4)
                )

            # ---- input DMAs (sync queue, deadline order) --------------------
            dma_x2n(0)
            dma_x2n(1)
            for k in range(kc):
                dma_x2T(k, 0)
            for k in range(kc):
                dma_x1T(k, 0)
            dma_x1n(0)
            dma_x1n(1)
            for k in range(kc):
                dma_x1T(k, 1)
            for g in range(2, g1):
                dma_x1n(g)
            for g in range(2, g2):
                dma_x2n(g)
            for cg in range(1, n_cgs):
                for k in range(kc):
                    dma_x2T(k, cg)

            # ---- PE warmup against the HAM clock gate -----------------------
            for w in range(30):
                ps_w = psb.tile([P, 512], F32, tag="psb", name=f"warm_{w}")
                nc.tensor.matmul(ps_w[:], lhsT=ones_h[:], rhs=warm[:],
                                 start=True, stop=True)

            # ---- stats / broadcast helpers ----------------------------------
            ssq2 = [stat.tile([P, 8], F32, tag=f"ssq2_{cg}", name=f"ssq2_{cg}")
                    for cg in range(n_cgs)]
            inv2 = [stat.tile([P, 8], F32, tag=f"inv2_{cg}", name=f"inv2_{cg}")
                    for cg in range(n_cgs)]
            ssq1 = stat.tile([P, 4 * g1], F32, tag="ssq1")
            inv1 = stat.tile([P, 4 * g1], F32, tag="inv1")
            dg4s = {}
            psbs = {}
            bcs = [bcp.tile([P, CG], BF16, tag=f"bc_{cg}", name=f"bc_{cg}")
                   for cg in range(n_cgs)]

            def sq_j(src, acc, j, eng="act"):
                """acc[:, j] = row sum of src[:, j]^2 on the given engine."""
                if eng == "act":
                    sq_t = sqp.tile([P, 512], BF16, tag="sqa")
                    nc.scalar.activation(
                        sq_t[:], src[:, j], AF.Square,
                        accum_out=acc[:, j:j + 1],
                    )
                else:
                    sq_t = sqp.tile([P, 512], BF16, tag="sqv")
                    nc.vector.tensor_mul(sq_t[:], src[:, j], src[:, j])
                    nc.vector.tensor_reduce(
                        acc[:, j:j + 1], sq_t[:], op=ALU.add,
                        axis=mybir.AxisListType.X,
                    )

            def stats_x2_group(cg, h):
                """ssq2[cg][:, 4h:4h+4] from x2n group 2*cg+h (split engines)."""
                g = 2 * cg + h
                s = ssq2[cg]
                for j in range(4):
                    sq_j(x2n_t[g], s[:, 4 * h:4 * h + 4], j,
                         "act" if j < 2 else "dve")

            def inv2_finish(cg, h=None):
                """inv2[cg][half] = 1 / max(row_norm, EPS)."""
                sl = slice(0, 8) if h is None else slice(4 * h, 4 * h + 4)
                iv = inv2[cg][:, sl]
                nc.scalar.activation(iv, ssq2[cg][:, sl], AF.Sqrt)
                nc.vector.tensor_scalar_max(iv, iv, EPS)
                nc.vector.reciprocal(iv, iv)

            def stats_x1_group(g, engs=("act", "act", "act", "act")):
                """inv1[:, 4g:4g+4] = 16 / max(row_norm, EPS) (scale folded)."""
                for j in range(4):
                    sq_j(x1n_t[g], ssq1[:, 4 * g:4 * g + 4], j, engs[j])
                sl = slice(4 * g, 4 * g + 4)
                nc.scalar.activation(inv1[:, sl], ssq1[:, sl], AF.Sqrt,
                                     scale=1.0 / 256.0)
                nc.vector.tensor_scalar_max(inv1[:, sl], inv1[:, sl],
                                            EPS / 16.0)
                nc.vector.reciprocal(inv1[:, sl], inv1[:, sl])

            def dg4_build(cg, h):
                dg4 = dgp.tile([P, 4, P], FP16, tag="dg", name=f"dg_{cg}_{h}")
                nc.vector.tensor_mul(
                    dg4[:], ident4[:],
                    inv2[cg][:, 4 * h:4 * h + 4, None].to_broadcast((P, 4, P)),
                )
                dg4s[(cg, h)] = dg4

            def bcast_mm(cg, h):
                ps_b = psb.tile([P, 512], F32, tag="psb", name=f"psb_{cg}_{h}")
                nc.tensor.matmul(ps_b[:], lhsT=ones_h[:], rhs=dg4s[(cg, h)][:],
                                 start=True, stop=True)
                psbs[(cg, h)] = ps_b

            def bc_copy(cg, h):
                c0 = 4 * h * P
                nc.scalar.activation(bcs[cg][:, c0:c0 + 512],
                                     psbs[(cg, h)][:], AF.Copy)

            def scale_x2(cg, k, h=None):
                """x2T[k][cg] *= bcs[cg] in place (DVE, baseline-proven)."""
                sl = slice(0, CG) if h is None else slice(512 * h,
                                                          512 * (h + 1))
                nc.vector.tensor_mul(x2T[k][cg][:, sl], x2T[k][cg][:, sl],
                                     bcs[cg][:, sl])

            def gemm_m(cg, m):
                pss = [psp.tile([P, 512], F32, tag="ps",
                                name=f"ps_{cg}_{m}_{j}") for j in range(2)]
                ot = otp.tile([P, CG], BF16, tag="ot", name=f"ot_{cg}_{m}")
                iv = inv1[:, m:m + 1]
                last = (cg == n_cgs - 1) and (m >= m_tiles - 2)
                # j-outer: finish bank j0's accumulation first, evict it on
                # DVE while bank j1's matmuls run, then evict j1 on ACT.
                # Each engine does one [P, 512] per m-tile -> banks recycle
                # ~2x faster than both-halves-on-one-engine.
                for j in range(2):
                    for k in range(kc):
                        nc.tensor.matmul(
                            pss[j][:],
                            lhsT=x1T[k][m // 16][:, (m % 16) * P:
                                                 (m % 16 + 1) * P],
                            rhs=x2T[k][cg][:, j * 512:(j + 1) * 512],
                            start=(k == 0), stop=(k == kc - 1),
                        )
                    sl = slice(j * 512, (j + 1) * 512)
                    if (j + m) % 2 == 0:
                        nc.vector.tensor_scalar_mul(ot[:, sl], pss[j][:], iv)
                    else:
                        nc.scalar.activation(ot[:, sl], pss[j][:],
                                             AF.Copy, scale=iv)
                if last:
                    base = cg * CG
                    nc.sync.dma_start(
                        out.ap()[m * P:(m + 1) * P, base:base + 512],
                        ot[:, 0:512])
                    nc.sync.dma_start(
                        out.ap()[m * P:(m + 1) * P, base + 512:base + 1024],
                        ot[:, 512:1024])
                else:
                    nc.sync.dma_start(
                        out.ap()[m * P:(m + 1) * P, cg * CG:(cg + 1) * CG],
                        ot[:])

            # ---- bootstrap stats for cg0 (overlaps warmup + first DMAs) -----
            # Program order defines RAW deps in the online Tile tracker, so
            # everything the cg0 evictions read (bcs[0], inv1 g0) must be
            # emitted before gemm_m(0, 0). The PE's gemm matmuls don't depend
            # on any of it, so the scheduler still overlaps them.
            stats_x2_group(0, 0)
            inv2_finish(0, 0)
            dg4_build(0, 0)
            bcast_mm(0, 0)
            bc_copy(0, 0)
            stats_x2_group(0, 1)
            inv2_finish(0, 1)
            dg4_build(0, 1)
            bcast_mm(0, 1)
            bc_copy(0, 1)
            for k in range(kc):
                scale_x2(0, k, 0)
            for k in range(kc):
                scale_x2(0, k, 1)
            stats_x1_group(0)

            # ---- main loop --------------------------------------------------
            # per cg: 16 m-tiles; bcast matmuls for cg0 slot in after m0 (their
            # diag inputs are ready by then); stats/broadcast/scale for cg+1
            # are spread through the window.
            for cg in range(n_cgs):
                nxt = cg + 1
                for m in range(m_tiles):
                    gemm_m(cg, m)
                    if cg == 0:
                        # x1 norm group g is needed by m-tile 4g's eviction;
                        # one group per ~7us keeps the engines unclustered.
                        if m == 0:
                            stats_x1_group(1)
                        elif m == 3:
                            stats_x2_group(1, 0)
                        elif m == 5:
                            stats_x2_group(1, 1)
                            inv2_finish(1)
                        elif m == 7:
                            dg4_build(1, 0)
                            dg4_build(1, 1)
                        elif m in (6, 10, 14, 18, 22, 26):
                            stats_x1_group(2 + (m - 6) // 4,
                                           ("act", "act", "dve", "dve")
                                           if m in (10, 18) else
                                           ("act", "act", "act", "act"))
                    if nxt < n_cgs:
                        if m == 25:
                            bcast_mm(nxt, 0)
                            bc_copy(nxt, 0)
                        elif m == 27:
                            bcast_mm(nxt, 1)
                            bc_copy(nxt, 1)
                            for k in range(kc):
                                scale_x2(nxt, k)

    nc.compile()
    return nc


def _get_program():
    key = "default"
    if key not in _PROGRAM_CACHE:
        _PROGRAM_CACHE[key] = build_program()
    return _PROGRAM_CACHE[key]


def _norm_groups(x8: np.ndarray) -> np.ndarray:
    """[G*512, 512] f32 -> [G, 128, 2048] fp8 with rows g*512+j*128+p."""
    g = x8.shape[0] // 512
    r = x8.reshape(g, 4, P, 512).transpose(0, 2, 1, 3).reshape(g, P, 2048)
    return np.ascontiguousarray(r.astype(ml_dtypes.float8_e4m3))


def make_in_maps(x1: np.ndarray, x2: np.ndarray) -> list:
    x1 = np.asarray(x1, dtype=np.float32)
    x2 = np.asarray(x2, dtype=np.float32)
    assert x1.shape == (N1, D) and x2.shape == (N2, D), (x1.shape, x2.shape)
    x1_b = x1.astype(ml_dtypes.bfloat16)
    x2_b = x2.astype(ml_dtypes.bfloat16)
    rows = N1 // GRID_I
    cols = N2 // GRID_J
    x1t_i = [np.ascontiguousarray(x1_b[i * rows:(i + 1) * rows].T)
             for i in range(GRID_I)]
    x1n_i = [_norm_groups(x1[i * rows:(i + 1) * rows]) for i in range(GRID_I)]
    x2t_j = [np.ascontiguousarray(x2_b[j * cols:(j + 1) * cols].T)
             for j in range(GRID_J)]
    x2n_j = [_norm_groups(x2[j * cols:(j + 1) * cols]) for j in range(GRID_J)]
    maps = []
    for c in range(N_CORES):
        i, j = c // GRID_J, c % GRID_J
        maps.append({
            "x1t": x1t_i[i],
            "x1n": x1n_i[i],
            "x2n": x2n_j[j],
            "x2t": x2t_j[j],
        })
    return maps


def kernel(x1: np.ndarray, x2: np.ndarray) -> np.ndarray:
    nc = _get_program()
    in_maps = make_in_maps(x1, x2)
    res = run_bass_kernel_spmd(nc, in_maps, core_ids=list(range(N_CORES)))
    rows = N1 // GRID_I
    cols = N2 // GRID_J
    full = np.empty((N1, N2), dtype=np.float32)
    for c in range(N_CORES):
        i, j = c // GRID_J, c % GRID_J
        full[i * rows:(i + 1) * rows, j * cols:(j + 1) * cols] = \
            res.results[c]["out"]
    return full


if __name__ == "__main__":
    rng = np.random.default_rng(0)
    a = rng.standard_normal((N1, D), dtype=np.float32)
    b = rng.standard_normal((N2, D), dtype=np.float32)
    got = kernel(a, b)
    n1 = np.maximum(np.linalg.norm(a, axis=-1, keepdims=True), EPS)
    n2 = np.maximum(np.linalg.norm(b, axis=-1, keepdims=True), EPS)
    want = SCALE * (a / n1) @ (b / n2).T
    err = np.abs(got - want)
    rel = np.linalg.norm(got - want) / np.linalg.norm(want)
    print(f"max abs err: {err.max():.3e}  rel: {rel:.3e}")


# revision 42
# speedup vs baseline: 1.0305x; 1.0178x over previous
"""Pairwise cosine-similarity kernel for Trainium2 (8 NeuronCores, SPMD).

Computes out = 16 * normalize(x1) @ normalize(x2).T for x1, x2 [8192, 512] f32.

Sharding: 2x4 grid. x1 rows split 2 ways (4096 rows/core), x2 rows split 4
ways (2048 rows/core); each core computes a [4096, 2048] output block; the
host assembles the grid and upcasts bf16 -> f32. Grid frontier (measured):
4x2 = 143.4us (engine-budget-bound), 2x4 = 142.7us (sweet spot), 1x8 =
157.8us (DMA-bound: 8MB x1t cannot stream past output-DMA competition).

Host-side prep is layout/dtype only:
  - x1t [512, 4096] bf16: pre-transposed x1 slice (GEMM stationary source),
    loaded as 8 x [128, 2048] SBUF tiles (the proven tile shape).
  - x2t [512, 2048] bf16: pre-transposed x2 slice (GEMM moving source).
  - x1n [8, 128, 2048] fp8e4m3, x2n [4, 128, 2048] fp8e4m3: natural-layout
    row-grouped copies used only for row-norm computation
    (group g holds rows g*512 + j*128 + p at [g, p, j*512:(j+1)*512]).

All FLOPs run on device. Schedule notes (HW-measured 142.7us; the
prior-session baseline was 154.7us):
  1. Input DMA is deadline-ordered on the sync HWDGE queue: x2n g0/g1
     (norm stats feed the x2t scale chain), then x2t cg0, then all of x1t,
     then the rest. First bytes land ~8.7us after NEFF start (the engine
     preamble runs to ~6.9us and DMA can't start earlier).
  2. 30 junk fp16 matmuls fill 7.9-16us purely to lift the PE HAM clock
     gate (1.2 -> 2.4 GHz) before real work; they overlap the input DMA.
     Real GEMM matmuls start ~22us, paced by the x2t scale chain.
  3. Norm math: squares split ACT (j0/j1, Square+accum) / DVE (j2/j3,
     tensor_mul + tensor_reduce; tensor_tensor_reduce kills the device).
     inv1 folds the 16x output scale; inv2 is partition-broadcast via fp16
     diag matmuls (ones.T @ diag(inv2)). Stats for cg N+1 are emitted just
     below cg N's first evictions in priority so eviction latency (which
     gates PSUM bank recycling) stays low.
  4. PSUM: 6+2 single-bank [128, 512] tiles. Matmuls targeting offset
     slices of a 2-bank [128, 1024] PSUM tensor run but drop the whole
     chip to ~5/6 clock (as does chunking x1t into [128, 512] tiles) --
     keep matmul dst = whole single-bank tiles and x1t as 4 x [128, 2048].
  5. Matmuls run j-outer per m-tile so PSUM bank j0 is evicted (one
     [128, 512] op) while bank j1's matmuls still run; the evicting engine
     alternates with (j + m) parity. Out-DMA triggers issue from the sync
     queue; the last two m-tiles split across both engines and two DMAs
     each. A dummy early Sqrt preloads the ACT table (else a lazy 1.3us
     ACT_TABLE_LOAD lands on the stats critical path).
  6. Measured-fragile: FIFTEEN variants across three grids (DMA reorders,
     queue reassignment, deferred rescale, stats re-placement, ot pool
     6->8) all ran slower than this exact configuration. Earlier-landing
     norm DMAs also regress (their stats ops then fire amid the evictions
     that gate PSUM recycling -- the 'late' DMA order is a natural
     throttle). Re-measure any change; CoreSim does not predict these.
"""

import sys

for _p in ("/root/.axon_site/_ro/trn_rl_repo", "/opt/trn_rl_repo"):
    if _p not in sys.path:
        sys.path.append(_p)

import ml_dtypes
import numpy as np

import concourse.bass as bass
import concourse.tile as tile
from concourse import bacc, mybir
from concourse.bass_utils import run_bass_kernel_spmd
from concourse.masks import make_identity

F32 = mybir.dt.float32
BF16 = mybir.dt.bfloat16
FP16 = mybir.dt.float16
FP8 = mybir.dt.float8e4
P = 128
SCALE = 16.0
EPS = 1e-8

N_CORES = 8
GRID_I = 2  # row-shards of x1
GRID_J = 4  # column-shards of x2
N1 = 8192
N2 = 8192
D = 512
CG = 1024  # output column-group width

_PROGRAM_CACHE = {}


def build_program(n1_local=N1 // GRID_I, n2=N2 // GRID_J, d=D):
    kc = d // P                 # 4 contraction chunks
    m_tiles = n1_local // P     # 16 row tiles per core
    n_cgs = n2 // CG            # 4 column groups
    g1 = n1_local // 512        # 4 x1 norm groups
    g2 = n2 // 512              # 8 x2 norm groups

    nc = bacc.Bacc("TRN2", target_bir_lowering=False, debug=False,
                   num_devices=N_CORES)
    x1t = nc.dram_tensor("x1t", [d, n1_local], BF16, kind="ExternalInput")
    x1n = nc.dram_tensor("x1n", [g1, P, 2048], FP8, kind="ExternalInput")
    x2n = nc.dram_tensor("x2n", [g2, P, 2048], FP8, kind="ExternalInput")
    x2t = nc.dram_tensor("x2t", [d, n2], BF16, kind="ExternalInput")
    out = nc.dram_tensor("out", [n1_local, n2], BF16, kind="ExternalOutput")

    AF = mybir.ActivationFunctionType
    ALU = mybir.AluOpType

    with tile.TileContext(nc) as tc:
        with (
            tc.tile_pool(name="const", bufs=1) as const,
            tc.tile_pool(name="xt", bufs=1) as xt,
            tc.tile_pool(name="ldn", bufs=1) as ldn,
            tc.tile_pool(name="sq", bufs=4) as sqp,
            tc.tile_pool(name="stat", bufs=1) as stat,
            tc.tile_pool(name="dg", bufs=2) as dgp,
            tc.tile_pool(name="bc", bufs=1) as bcp,
            tc.tile_pool(name="ot", bufs=6) as otp,
            tc.tile_pool(name="ps", bufs=6, space="PSUM") as psp,
            tc.tile_pool(name="psb", bufs=2, space="PSUM") as psb,
        ):
            # ---- constants (cheap memsets so PE warmup starts early) --------
            ones_h = const.tile([P, P], FP16)
            nc.gpsimd.memset(ones_h[:], 1.0)
            warm = const.tile([P, 512], FP16)
            nc.gpsimd.memset(warm[:], 0.0)
            ident4 = const.tile([P, 4, P], FP16)
            nc.gpsimd.memset(ident4[:], 0.0)
            for b in range(4):
                make_identity(nc, ident4[:, b], nomemset=True)
            # dummy sqrt pulls the ACT_TABLE_LOAD for Sqrt off the stats
            # critical path (it otherwise loads lazily, 1.3us, right before
            # the first inv2 sqrt).
            tblw = const.tile([P, 1], F32)
            nc.scalar.activation(tblw[:], ones_h[:, 0:1], AF.Sqrt)

            # ---- SBUF input tiles -------------------------------------------
            x1n_t = [ldn.tile([P, 4, 512], FP8, tag=f"x1n_{g}",
                              name=f"x1n_{g}") for g in range(g1)]
            x2n_t = [ldn.tile([P, 4, 512], FP8, tag=f"x2n_{g}",
                              name=f"x2n_{g}") for g in range(g2)]
            n_c = n1_local // 2048
            x1T = [[xt.tile([P, 2048], BF16, tag=f"x1T_{k}_{c}",
                            name=f"x1T_{k}_{c}") for c in range(n_c)]
                   for k in range(kc)]
            x2T = [[xt.tile([P, CG], BF16, tag=f"x2T_{k}_{cg}",
                            name=f"x2T_{k}_{cg}") for cg in range(n_cgs)]
                   for k in range(kc)]

            def dma_x2T(k, cg, eng=None):
                (eng or nc.sync).dma_start(
                    x2T[k][cg][:], x2t[k * P:(k + 1) * P,
                                       cg * CG:(cg + 1) * CG])

            def dma_x1T(k, c):
                nc.sync.dma_start(
                    x1T[k][c][:], x1t[k * P:(k + 1) * P,
                                      c * 2048:(c + 1) * 2048])

            def dma_x2n(g, eng=None):
                (eng or nc.sync).dma_start(
                    x2n_t[g][:], x2n.ap()[g].rearrange("p (j e) -> p j e", j=4)
                )

            def dma_x1n(g):
                nc.sync.dma_start(
                    x1n_t[g][:], x1n.ap()[g].rearrange("p (j e) -> p j e", j=4)
                )

            # ---- input DMAs (sync queue, deadline order) --------------------
            dma_x2n(0)
            dma_x2n(1)
            for k in range(kc):
                dma_x2T(k, 0)
            for k in range(kc):
                dma_x1T(k, 0)
            dma_x1n(0)
            dma_x1n(1)
            for k in range(kc):
                dma_x1T(k, 1)
            for g in range(2, g1):
                dma_x1n(g)
            for g in range(2, g2):
                dma_x2n(g)
            for cg in range(1, n_cgs):
                for k in range(kc):
                    dma_x2T(k, cg)

            # ---- PE warmup against the HAM clock gate -----------------------
            for w in range(30):
                ps_w = psb.tile([P, 512], F32, tag="psb", name=f"warm_{w}")
                nc.tensor.matmul(ps_w[:], lhsT=ones_h[:], rhs=warm[:],
                                 start=True, stop=True)

            # ---- stats / broadcast helpers ----------------------------------
            ssq2 = [stat.tile([P, 8], F32, tag=f"ssq2_{cg}", name=f"ssq2_{cg}")
                    for cg in range(n_cgs)]
            inv2 = [stat.tile([P, 8], F32, tag=f"inv2_{cg}", name=f"inv2_{cg}")
                    for cg in range(n_cgs)]
            ssq1 = stat.tile([P, 4 * g1], F32, tag="ssq1")
            inv1 = stat.tile([P, 4 * g1], F32, tag="inv1")
            dg4s = {}
            psbs = {}
            bcs = [bcp.tile([P, CG], BF16, tag=f"bc_{cg}", name=f"bc_{cg}")
                   for cg in range(n_cgs)]

            def sq_j(src, acc, j, eng="act"):
                """acc[:, j] = row sum of src[:, j]^2 on the given engine."""
                if eng == "act":
                    sq_t = sqp.tile([P, 512], BF16, tag="sqa")
                    nc.scalar.activation(
                        sq_t[:], src[:, j], AF.Square,
                        accum_out=acc[:, j:j + 1],
                    )
                else:
                    sq_t = sqp.tile([P, 512], BF16, tag="sqv")
                    nc.vector.tensor_mul(sq_t[:], src[:, j], src[:, j])
                    nc.vector.tensor_reduce(
                        acc[:, j:j + 1], sq_t[:], op=ALU.add,
                        axis=mybir.AxisListType.X,
                    )

            def stats_x2_group(cg, h):
                """ssq2[cg][:, 4h:4h+4] from x2n group 2*cg+h (split engines)."""
                g = 2 * cg + h
                s = ssq2[cg]
                for j in range(4):
                    sq_j(x2n_t[g], s[:, 4 * h:4 * h + 4], j,
                         "act" if j < 2 else "dve")

            def inv2_finish(cg, h=None):
                """inv2[cg][half] = 1 / max(row_norm, EPS)."""
                sl = slice(0, 8) if h is None else slice(4 * h, 4 * h + 4)
                iv = inv2[cg][:, sl]
                nc.scalar.activation(iv, ssq2[cg][:, sl], AF.Sqrt)
                nc.vector.tensor_scalar_max(iv, iv, EPS)
                nc.vector.reciprocal(iv, iv)

            def stats_x1_group(g, engs=("act", "act", "act", "act")):
                """inv1[:, 4g:4g+4] = 16 / max(row_norm, EPS) (scale folded)."""
                for j in range(4):
                    sq_j(x1n_t[g], ssq1[:, 4 * g:4 * g + 4], j, engs[j])
                sl = slice(4 * g, 4 * g + 4)
                nc.scalar.activation(inv1[:, sl], ssq1[:, sl], AF.Sqrt,
                                     scale=1.0 / 256.0)
                nc.vector.tensor_scalar_max(inv1[:, sl], inv1[:, sl],
                                            EPS / 16.0)
                nc.vector.reciprocal(inv1[:, sl], inv1[:, sl])

            def dg4_build(cg, h):
                dg4 = dgp.tile([P, 4, P], FP16, tag="dg", name=f"dg_{cg}_{h}")
                nc.vector.tensor_mul(
                    dg4[:], ident4[:],
                    inv2[cg][:, 4 * h:4 * h + 4, None].to_broadcast((P, 4, P)),
                )
                dg4s[(cg, h)] = dg4

            def bcast_mm(cg, h):
                ps_b = psb.tile([P, 512], F32, tag="psb", name=f"psb_{cg}_{h}")
                nc.tensor.matmul(ps_b[:], lhsT=ones_h[:], rhs=dg4s[(cg, h)][:],
                                 start=True, stop=True)
                psbs[(cg, h)] = ps_b

            def bc_copy(cg, h):
                c0 = 4 * h * P
                nc.scalar.activation(bcs[cg][:, c0:c0 + 512],
                                     psbs[(cg, h)][:], AF.Copy)

            def scale_x2(cg, k, h=None):
                """x2T[k][cg] *= bcs[cg] in place (DVE, baseline-proven)."""
                sl = slice(0, CG) if h is None else slice(512 * h,
                                                          512 * (h + 1))
                nc.vector.tensor_mul(x2T[k][cg][:, sl], x2T[k][cg][:, sl],
                                     bcs[cg][:, sl])

            def gemm_m(cg, m):
                pss = [psp.tile([P, 512], F32, tag="ps",
                                name=f"ps_{cg}_{m}_{j}") for j in range(2)]
                ot = otp.tile([P, CG], BF16, tag="ot", name=f"ot_{cg}_{m}")
                iv = inv1[:, m:m + 1]
                last = (cg == n_cgs - 1) and (m >= m_tiles - 2)
                # j-outer: finish bank j0's accumulation first, evict it on
                # DVE while bank j1's matmuls run, then evict j1 on ACT.
                # Each engine does one [P, 512] per m-tile -> banks recycle
                # ~2x faster than both-halves-on-one-engine.
                for j in range(2):
                    for k in range(kc):
                        nc.tensor.matmul(
                            pss[j][:],
                            lhsT=x1T[k][m // 16][:, (m % 16) * P:
                                                 (m % 16 + 1) * P],
                            rhs=x2T[k][cg][:, j * 512:(j + 1) * 512],
                            start=(k == 0), stop=(k == kc - 1),
                        )
                    sl = slice(j * 512, (j + 1) * 512)
                    if (j + m) % 2 == 0:
                        nc.vector.tensor_scalar_mul(ot[:, sl], pss[j][:], iv)
                    else:
                        nc.scalar.activation(ot[:, sl], pss[j][:],
                                             AF.Copy, scale=iv)
                if last:
                    base = cg * CG
                    nc.sync.dma_start(
                        out.ap()[m * P:(m + 1) * P, base:base + 512],
                        ot[:, 0:512])
                    nc.sync.dma_start(
                        out.ap()[m * P:(m + 1) * P, base + 512:base + 1024],
                        ot[:, 512:1024])
                else:
                    nc.sync.dma_start(
                        out.ap()[m * P:(m + 1) * P, cg * CG:(cg + 1) * CG],
                        ot[:])

            # ---- bootstrap stats for cg0 (overlaps warmup + first DMAs) -----
            # Program order defines RAW deps in the online Tile tracker, so
            # everything the cg0 evictions read (bcs[0], inv1 g0) must be
            # emitted before gemm_m(0, 0). The PE's gemm matmuls don't depend
            # on any of it, so the scheduler still overlaps them.
            stats_x2_group(0, 0)
            inv2_finish(0, 0)
            dg4_build(0, 0)
            bcast_mm(0, 0)
            bc_copy(0, 0)
            stats_x2_group(0, 1)
            inv2_finish(0, 1)
            dg4_build(0, 1)
            bcast_mm(0, 1)
            bc_copy(0, 1)
            for k in range(kc):
                scale_x2(0, k, 0)
            for k in range(kc):
                scale_x2(0, k, 1)
            stats_x1_group(0)

            # ---- main loop --------------------------------------------------
            # per cg: 16 m-tiles; bcast matmuls for cg0 slot in after m0 (their
            # diag inputs are ready by then); stats/broadcast/scale for cg+1
            # are spread through the window.
            for cg in range(n_cgs):
                nxt = cg + 1
                for m in range(m_tiles):
                    gemm_m(cg, m)
                    if cg == 0:
                        # x1 norm group g is needed by m-tile 4g's eviction;
                        # one group per ~7us keeps the engines unclustered.
                        if m == 0:
                            stats_x1_group(1)
                        elif m == 3:
                            stats_x2_group(1, 0)
                        elif m == 5:
                            stats_x2_group(1, 1)
                            inv2_finish(1)
                        elif m == 7:
                            dg4_build(1, 0)
                            dg4_build(1, 1)
                        elif m in (6, 10, 14, 18, 22, 26):
                            stats_x1_group(2 + (m - 6) // 4,
                                           ("act", "act", "dve", "dve")
                                           if m in (10, 18) else
                                           ("act", "act", "act", "act"))
                    if nxt < n_cgs:
                        if m == 25:
                            bcast_mm(nxt, 0)
                            bc_copy(nxt, 0)
                        elif m == 27:
                            bcast_mm(nxt, 1)
                            bc_copy(nxt, 1)
                            for k in range(kc):
                                scale_x2(nxt, k)

    nc.compile()
    return nc


def _get_program():
    key = "default"
    if key not in _PROGRAM_CACHE:
        _PROGRAM_CACHE[key] = build_program()
    return _PROGRAM_CACHE[key]


def _norm_groups(x8: np.ndarray) -> np.ndarray:
    """[G*512, 512] f32 -> [G, 128, 2048] fp8 with rows g*512+j*128+p."""
    g = x8.shape[0] // 512
    r = x8.reshape(g, 4, P, 512).transpose(0, 2, 1, 3).reshape(g, P, 2048)
    return np.ascontiguousarray(r.astype(ml_dtypes.float8_e4m3))


def make_in_maps(x1: np.ndarray, x2: np.ndarray) -> list:
    x1 = np.asarray(x1, dtype=np.float32)
    x2 = np.asarray(x2, dtype=np.float32)
    assert x1.shape == (N1, D) and x2.shape == (N2, D), (x1.shape, x2.shape)
    x1_b = x1.astype(ml_dtypes.bfloat16)
    x2_b = x2.astype(ml_dtypes.bfloat16)
    rows = N1 // GRID_I
    cols = N2 // GRID_J
    x1t_i = [np.ascontiguousarray(x1_b[i * rows:(i + 1) * rows].T)
             for i in range(GRID_I)]
    x1n_i = [_norm_groups(x1[i * rows:(i + 1) * rows]) for i in range(GRID_I)]
    x2t_j = [np.ascontiguousarray(x2_b[j * cols:(j + 1) * cols].T)
             for j in range(GRID_J)]
    x2n_j = [_norm_groups(x2[j * cols:(j + 1) * cols]) for j in range(GRID_J)]
    maps = []
    for c in range(N_CORES):
        i, j = c // GRID_J, c % GRID_J
        maps.append({
            "x1t": x1t_i[i],
            "x1n": x1n_i[i],
            "x2n": x2n_j[j],
            "x2t": x2t_j[j],
        })
    return maps


def kernel(x1: np.ndarray, x2: np.ndarray) -> np.ndarray:
    nc = _get_program()
    in_maps = make_in_maps(x1, x2)
    res = run_bass_kernel_spmd(nc, in_maps, core_ids=list(range(N_CORES)))
    rows = N1 // GRID_I
    cols = N2 // GRID_J
    full = np.empty((N1, N2), dtype=np.float32)
    for c in range(N_CORES):
        i, j = c // GRID_J, c % GRID_J
        full[i * rows:(i + 1) * rows, j * cols:(j + 1) * cols] = \
            res.results[c]["out"]
    return full


if __name__ == "__main__":
    rng = np.random.default_rng(0)
    a = rng.standard_normal((N1, D), dtype=np.float32)
    b = rng.standard_normal((N2, D), dtype=np.float32)
    got = kernel(a, b)
    n1 = np.maximum(np.linalg.norm(a, axis=-1, keepdims=True), EPS)
    n2 = np.maximum(np.linalg.norm(b, axis=-1, keepdims=True), EPS)
    want = SCALE * (a / n1) @ (b / n2).T
    err = np.abs(got - want)
    rel = np.linalg.norm(got - want) / np.linalg.norm(want)
    print(f"max abs err: {err.max():.3e}  rel: {rel:.3e}")
